# revision 117
# baseline (speedup 1.0000x reference)
"""Trainium2 Bass kernel for nn_Attention (B=4, N=2048, C=768, H=12, D=64).

Sharding: core c -> batch b=c//2, head-group hg=c%2 (6 heads each).
qkv_w column-parallel, proj_w row-parallel (host sums the 2 partials per b).

v2 design (cost-model driven):
  - All matmul operands bf16 (1.0 cycles/row on PE, same as fp32r, but half
    the DMA bytes and SBUF, and 1.0 c/r transposes).
  - AV uses es as the STATIONARY operand: out = [128 qi, 65] per (qi-tile,
    head) accumulating over 16 ki tiles -> 65 rows/matmul with full 128
    output partitions (vs 512 rows with 65 partitions when es moves).
    Saves ~97k PE cycles/core.  The attention output lands token-major;
    a cheap PE re-transpose (128 rows per 2-head block) feeds the
    feature-major projection.
  - Softmax denominator from a ones-column in vA (65th feature); normalize
    with a per-token reciprocal broadcast along the free dim (no gpsimd
    partition_broadcast).
  - exp on ACT is the hard floor (~199us); all copies are kept off ACT in
    the attention phases and prep work is interleaved into the per-kpair
    ACT-gated gaps of the PE stream.
"""
import sys

sys.path.insert(0, "/opt/trn_rl_repo")

import numpy as np
import ml_dtypes
import concourse.bass as bass
import concourse.mybir as mybir
import concourse.tile as tile
from concourse import bacc
from concourse.bass_utils import run_bass_kernel_spmd
from concourse.masks import make_identity

dt = mybir.dt
AF = mybir.ActivationFunctionType
ALU = mybir.AluOpType
AX = mybir.AxisListType

B, N, C = 4, 2048, 768
H, D = 12, 64
HPC = 6            # heads per core
EPS = 1e-6
NT = N // 128      # 16 token tiles
NCHUNK = C // 128  # 6 contraction chunks
SCALE = D ** -0.5  # 0.125
NG = 4             # qi groups
G = N // NG        # 512 per group
ND = NT // 2       # 8 duos (2 token tiles per prep step)

BF = dt.bfloat16


def _bc(ap, idx, count):
    """Insert a broadcast (step 0) free dim at position idx of an AP."""
    a = list(ap.ap)
    a.insert(idx, [0, count])
    return bass.AP(tensor=ap.tensor, offset=ap.offset, ap=a)


def _swap_halves(ap):
    """View with the two halves of the last free dim swapped (as [2, h])."""
    a = [list(d) for d in ap.ap]
    st, cnt = a[-1]
    h = cnt // 2
    a = a[:-1] + [[-st * h, 2], [st, h]]
    return bass.AP(tensor=ap.tensor, offset=ap.offset + st * h, ap=a)


def _split_last(ap, u):
    """Split the last free dim [st, cnt] into [u, cnt//u] (contiguous split)."""
    a = [list(d) for d in ap.ap]
    st, cnt = a[-1]
    v = cnt // u
    a = a[:-1] + [[st * v, u], [st, v]]
    return bass.AP(tensor=ap.tensor, offset=ap.offset, ap=a)


def _bcast_last(ap, count):
    """Replace a trailing [_, 1] free dim with a [0, count] broadcast dim."""
    a = [list(d) for d in ap.ap]
    assert a[-1][1] == 1
    a[-1] = [0, count]
    return bass.AP(tensor=ap.tensor, offset=ap.offset, ap=a)


def build_program():
    nc = bacc.Bacc(None, target_bir_lowering=False)

    xT = nc.dram_tensor("xT", [C, N], BF, kind="ExternalInput")
    # host layout: [q0|k0 (256) | v (384) | q1|k1 | q2|k2]
    wqkvT = nc.dram_tensor("wqkvT", [C, 3 * HPC * D], BF, kind="ExternalInput")
    projT = nc.dram_tensor("projT", [HPC * D, C], BF, kind="ExternalInput")
    cqk = nc.dram_tensor("cqk", [N, 2 * D], dt.float32, kind="ExternalInput")
    sqk = nc.dram_tensor("sqk", [N, 2 * D], dt.float32, kind="ExternalInput")
    out = nc.dram_tensor("out", [N, C], BF, kind="ExternalOutput")

    with tile.TileContext(nc) as tc:
        with (
            tc.tile_pool(name="persist", bufs=1) as persist,
            tc.tile_pool(name="qkrot", bufs=2) as qkrot,     # qT/kT rotate across pairs
            tc.tile_pool(name="work", bufs=4) as work,
            tc.tile_pool(name="qkblk", bufs=4) as qkblk,
            tc.tile_pool(name="tiny", bufs=4) as tiny,
            tc.tile_pool(name="osb", bufs=6) as osb,
            tc.tile_pool(name="outp", bufs=4) as outp,
            tc.tile_pool(name="p2e", bufs=12) as p2e,
            tc.tile_pool(name="psA", bufs=2, space="PSUM") as psA,   # qkv/tp/proj
            tc.tile_pool(name="psS", bufs=2, space="PSUM") as psS,   # scores
            tc.tile_pool(name="psV", bufs=2, space="PSUM") as psV,   # AV accum
        ):
            # ---------------- persistent tiles --------------------------------
            oT = [[persist.tile([128, G], BF, name=f"oT{p}_{g}", tag=f"oT{p}_{g}")
                   for g in range(NG)] for p in range(3)]
            vA = [persist.tile([128, 4, HPC, D + 1], BF, name=f"vA{kg}", tag=f"vA{kg}")
                  for kg in range(NG)]
            ident = persist.tile([128, 128], BF, tag="ident")
            make_identity(nc, ident[:])
            for kg in range(NG):
                nc.vector.memset(vA[kg][:, :, :, D : D + 1], 1.0)
            # PE p-state warm-up: ~3us of dummy transposes during the DMA ramp
            # so the first real matmuls run at 2.4GHz instead of 0.65-1.2GHz
            for _ in range(28):
                wtp = psA.tile([128, 128], BF, tag="qkv")
                nc.tensor.transpose(wtp[:], ident[:], ident[:])

            # weights / x^T / tables.  Two batched DMAs cover everything the
            # first prep duo needs; table halves follow so duo-0's RoPE isn't
            # starved; the rest streams in behind.
            wqkv_r = wqkvT.rearrange("(j p) f -> p j f", p=128)
            xT_r = xT.rearrange("(j p) n -> p j n", p=128)
            # startup-critical loads split so the first V matmuls (chunks 0-2
            # of duo 0) can start as early as possible
            wrA = persist.tile([128, NCHUNK, 640], BF, name="wrA", tag="wrA")
            xr0 = persist.tile([128, NCHUNK, G], BF, name="xr0", tag="xr0")
            nc.sync.dma_start(wrA[:, 0:3, 0:256], wqkv_r[:, 0:3, 0:256])   # qk first
            nc.sync.dma_start(xr0[:, 0:3, :], xT_r[:, 0:3, 0:G])
            nc.sync.dma_start(wrA[:, 3:6, 0:256], wqkv_r[:, 3:6, 0:256])
            nc.sync.dma_start(xr0[:, 3:6, :], xT_r[:, 3:6, 0:G])
            tabs = {}
            for name, dram in (("cqk", cqk), ("sqk", sqk)):
                tabs[name] = persist.tile([128, NT, 2, D], dt.float32, name=name, tag=name)
            tab_r = {name: dram.rearrange("(t p) (qk d) -> p t qk d", p=128, qk=2)
                     for name, dram in (("cqk", cqk), ("sqk", sqk))}
            nc.sync.dma_start(tabs["cqk"][:, 0:8, :, :], tab_r["cqk"][:, 0:8, :, :])
            nc.sync.dma_start(wrA[:, :, 256:640], wqkv_r[:, :, 256:640])   # V weights
            nc.sync.dma_start(tabs["sqk"][:, 0:8, :, :], tab_r["sqk"][:, 0:8, :, :])
            xrg = []
            for tg in range(1, NG):
                xg = persist.tile([128, NCHUNK, G], BF, name=f"xrg{tg}", tag=f"xrg{tg}")
                nc.sync.dma_start(xg[:], xT_r[:, :, tg * G : (tg + 1) * G])
                xrg.append(xg)
                if tg == 1:
                    for name in ("cqk", "sqk"):
                        nc.sync.dma_start(tabs[name][:, 8:16, :, :], tab_r[name][:, 8:16, :, :])
            wrB = persist.tile([128, NCHUNK, 512], BF, name="wrB", tag="wrB")
            nc.sync.dma_start(wrB[:], wqkv_r[:, :, 640:1152])
            prW = persist.tile([128, 3, C], BF, name="prW", tag="prW")
            nc.sync.dma_start(prW[:], projT.rearrange("(p q) f -> q p f", q=128))

            def xsl(j, tg, col0, col1):
                """x chunk j, token-group tg, token columns [col0:col1)."""
                if tg == 0:
                    return xr0[:, j, col0:col1]
                return xrg[tg - 1][:, j, col0:col1]

            # ------- prep: qkv matmuls + RMSNorm + RoPE, 2 token tiles/step ----
            def emit_v_duo(d, on_dve=False):
                i = 2 * d
                tg, c0 = i // NG, (i % NG) * 128
                for t in range(2):
                    vp = psA.tile([128, HPC * D], dt.float32, tag="qkv")
                    for j in range(NCHUNK):
                        nc.tensor.matmul(vp[:], xsl(j, tg, c0 + t * 128, c0 + (t + 1) * 128),
                                         wrA[:, j, 256:640],
                                         start=(j == 0), stop=(j == NCHUNK - 1))
                    # gpsimd cannot read PSUM; ACT is idle during early prep-0
                    # but exp-saturated once the gated units ramp, so deferred
                    # duos copy on DVE instead
                    vdst = vA[tg][:, i % NG + t, :, 0:D]
                    vsrc = vp[:].rearrange("p (h d) -> p h d", h=HPC)
                    if on_dve:
                        nc.vector.tensor_copy(vdst, vsrc)
                    else:
                        nc.scalar.copy(vdst, vsrc)

            def new_pair_state(p):
                # qkT[g]: columns 0:G hold q^T for qi-group g, G:2G hold k^T
                # for ki-group g.
                return {
                    "p": p,
                    "qkT": [qkrot.tile([128, 2 * G], BF, name=f"qkT{p}_{g}", tag=f"qkT{g}") for g in range(NG)],
                    "pend": [],
                    "next": 0,
                }

            def flush_one(st):
                d, qn2 = st["pend"].pop(0)
                i = 2 * d
                tp = psA.tile([128, 2, 256], BF, tag="qkv")
                for t in range(2):
                    nc.tensor.transpose(tp[:, t, 0:128], qn2[:, t, 0:128], ident[:])
                    nc.tensor.transpose(tp[:, t, 128:256], qn2[:, t, 128:256], ident[:])
                # tp[:, t, 0:128]=q^T(tile i+t), [:, t, 128:256]=k^T(tile i+t)
                g = i // NG
                c0 = (i % NG) * 128
                dst = st["qkT"][g][:, :].rearrange("p (qk c) -> p qk c", qk=2)[
                    :, :, c0 : c0 + 256].rearrange("p qk (t c) -> p qk t c", t=2)
                src = tp[:, :].rearrange("p t (qk c) -> p qk t c", qk=2)
                nc.vector.tensor_copy(dst, src)

            def emit_prep_duo(st):
                p = st["p"]
                d = st["next"]
                st["next"] += 1
                i = 2 * d
                tg, c0 = i // NG, (i % NG) * 128
                # Q/K projection first: it feeds the norm/RoPE chain that gates
                # the flushes (and therefore the exp stream); V matmuls follow.
                qkp = psA.tile([128, 2, 256], dt.float32, tag="qkv")
                for t in range(2):
                    for j in range(NCHUNK):
                        wsl = wrA[:, j, 0:256] if p == 0 else wrB[:, j, (p - 1) * 256 : p * 256]
                        nc.tensor.matmul(qkp[:, t, :], xsl(j, tg, c0 + t * 128, c0 + (t + 1) * 128),
                                         wsl, start=(j == 0), stop=(j == NCHUNK - 1))
                qk_sb = qkblk.tile([128, 2, 256], dt.float32, tag="qk_sb")
                if p == 0:
                    nc.scalar.copy(qk_sb[:], qkp[:])
                else:
                    nc.vector.tensor_copy(qk_sb[:], qkp[:])
                if p == 0 and d < 4:
                    emit_v_duo(d)
                if len(st["pend"]) >= 2:
                    flush_one(st)
                qk4 = qk_sb[:].rearrange("p t (h d) -> p (t h) d", h=4)   # [128, 8, 64]
                # rotation first (no norm factor needed — RMSNorm commutes with
                # the per-token rotation): m1/m2 run on Pool concurrently with
                # the DVE sum-of-squares/rsqrt chain.
                t4 = qk_sb[:].rearrange("p t (qk h d) -> p t qk h d", qk=2, h=2)
                cwb = _bc(tabs["cqk"][:, i : i + 2, :, :], 3, 2)
                swb = _bc(tabs["sqk"][:, i : i + 2, :, :], 3, 2)
                m1 = work.tile([128, 2, 2, 2, D], dt.float32, tag="m1")
                nc.vector.tensor_tensor(m1[:], t4, cwb, op=ALU.mult)
                m2 = work.tile([128, 2, 2, 2, D], dt.float32, tag="m2")
                nc.gpsimd.tensor_tensor(_split_last(m2[:], 2), _swap_halves(t4),
                                        _split_last(swb, 2), op=ALU.mult)
                sq = work.tile([128, 8, D], dt.float32, tag="sq")
                nc.vector.tensor_tensor(sq[:], qk4, qk4, op=ALU.mult)
                ss = tiny.tile([128, 8], dt.float32, tag="ss")
                nc.vector.tensor_reduce(ss[:], sq[:], axis=AX.X, op=ALU.add)
                # rsqrt on DVE (bit-trick + 2 Newton): nf = 1/sqrt(ss+D*EPS)
                ssh = tiny.tile([128, 8], dt.float32, tag="ssh")
                nc.vector.tensor_scalar(ssh[:], ss[:], 0.5, 0.5 * D * EPS,
                                        op0=ALU.mult, op1=ALU.add)
                y0i = tiny.tile([128, 8], dt.int32, tag="y0i")
                nc.vector.tensor_scalar(y0i[:], ss[:].bitcast(dt.int32), 1, 0,
                                        op0=ALU.logical_shift_right, op1=ALU.bitwise_or)
                nc.vector.tensor_scalar(y0i[:], y0i[:], -1, 0x5F3759DF,
                                        op0=ALU.mult, op1=ALU.add)
                nf = tiny.tile([128, 8], dt.float32, tag="nf")
                y1 = tiny.tile([128, 8], dt.float32, tag="y1")
                yw = tiny.tile([128, 8], dt.float32, tag="yw")
                y = y0i[:].bitcast(dt.float32)
                for dst_ in (y1, nf):
                    nc.vector.tensor_tensor(yw[:], y, y, op=ALU.mult)
                    nc.vector.tensor_tensor(yw[:], yw[:], ssh[:], op=ALU.mult)
                    nc.vector.tensor_scalar(yw[:], yw[:], -1.0, 1.5,
                                            op0=ALU.mult, op1=ALU.add)
                    nc.vector.tensor_tensor(dst_[:], y, yw[:], op=ALU.mult)
                    y = dst_[:]
                nfb = _bc(nf[:], 2, D)                                   # [128, 8, bc D]
                rsum = work.tile([128, 8, D], dt.float32, tag="rsum")
                nc.gpsimd.tensor_tensor(rsum[:].rearrange("p (t qk h) d -> p t qk h d", t=2, qk=2),
                                        m1[:], m2[:], op=ALU.add)
                qn2 = work.tile([128, 2, 256], BF, tag="qn2", bufs=3)
                nc.vector.tensor_tensor(qn2[:].rearrange("p t (f d) -> p (t f) d", f=4),
                                        rsum[:], nfb, op=ALU.mult)
                st["pend"].append((d, qn2))

            def finish_prep(st):
                while st["next"] < ND:
                    emit_prep_duo(st)
                while st["pend"]:
                    flush_one(st)

            # ---------------- attention unit (head h, qi-group g) --------------
            def att_unit_gen(st, g, hh, osb_t, qt_done=None):
                p = st["p"]
                h = 2 * p + hh
                off = 64 * hh
                # full-bank tile (2KB) so the zero region is private; ONE psum
                # accumulation group per unit: start only on the very first
                # matmul (its pending-zero marks the whole bank, so each qt
                # slice is zeroed on first touch), stop only on the last.
                av = psV.tile([128, 4, 128], dt.float32, tag="av")

                def emit_av(kpair, es):
                    for half in range(2):
                        ki = kpair * 2 + half
                        for qt in range(4):
                            nc.tensor.matmul(
                                av[:, qt, 0 : D + 1],
                                es[:, half * 512 + qt * 128 : half * 512 + (qt + 1) * 128],
                                vA[ki // NG][:, ki % NG, h, :],
                                start=(ki == 0 and qt == 0),
                                stop=(ki == NT - 1 and qt == 3),
                            )

                # AV trails exp by two kpairs so PE never stalls on the
                # in-flight ACTIVATE (es pool bufs=4 keeps 3 outstanding).
                prev = []
                for kpair in range(8):
                    sp = psS.tile([128, 1024], dt.float32, tag="sp")
                    for half in range(2):
                        ki = kpair * 2 + half
                        nc.tensor.matmul(
                            sp[:, half * 512 : (half + 1) * 512],
                            st["qkT"][ki // NG][off : off + 64, G + (ki % NG) * 128 : G + (ki % NG + 1) * 128],
                            st["qkT"][g][off : off + 64, 0:G],
                            start=True, stop=True,
                        )
                    es = p2e.tile([128, 1024], BF, name="est", tag="es")
                    nc.scalar.activation(es[:], sp[:], AF.Exp, scale=SCALE)
                    prev.append((kpair, es))
                    if len(prev) > 4:
                        emit_av(*prev.pop(0))
                    yield
                for pr in prev:
                    emit_av(*pr)
                for qt in range(4):
                    rd = tiny.tile([128, 1], dt.float32, tag="rd")
                    nc.vector.reciprocal(rd[:], av[:, qt, D : D + 1])
                    nc.vector.tensor_tensor(
                        osb_t[:, qt, off : off + 64],
                        av[:, qt, 0:D], _bcast_last(rd[:], D), op=ALU.mult,
                    )
                    if qt_done is not None:
                        qt_done(qt)

            def emit_otr(p, g, osb_t, qt):
                tp2 = psA.tile([128, 128], BF, tag="qkv")
                nc.tensor.transpose(tp2[:], osb_t[:, qt, :], ident[:])
                nc.vector.tensor_copy(oT[p][g][:, qt * 128 : (qt + 1) * 128], tp2[:])

            def emit_proj_tile(i, tail=False):
                p512 = psA.tile([128, 512], dt.float32, tag="qkv")
                p256 = psA.tile([128, 256], dt.float32, tag="qkv")
                for pp_ in range(3):
                    st_, spp = (pp_ == 0), (pp_ == 2)
                    sl = oT[pp_][i // NG][:, (i % NG) * 128 : (i % NG + 1) * 128]
                    nc.tensor.matmul(p512[:], sl, prW[:, pp_, 0:512], start=st_, stop=spp)
                os_ = outp.tile([128, C], BF, tag="os")
                nc.vector.tensor_copy(os_[:, 0:512], p512[:])
                for pp_ in range(3):
                    st_, spp = (pp_ == 0), (pp_ == 2)
                    sl = oT[pp_][i // NG][:, (i % NG) * 128 : (i % NG + 1) * 128]
                    nc.tensor.matmul(p256[:], sl, prW[:, pp_, 512:768], start=st_, stop=spp)
                if tail:
                    nc.scalar.copy(os_[:, 512:768], p256[:])   # ACT idle at tail
                else:
                    nc.vector.tensor_copy(os_[:, 512:768], p256[:])
                nc.sync.dma_start(out[i * 128 : (i + 1) * 128, :], os_[:])

            # ------------- schedule: prep0 with gated units (0,0,0/1) ----------
            cur = new_pair_state(0)
            osb_first = osb.tile([128, 4, 128], BF, tag="osb")
            osb_g1 = osb.tile([128, 4, 128], BF, tag="osb")
            gen0a = att_unit_gen(cur, 0, 0, osb_first)
            gen0b = att_unit_gen(cur, 0, 1, osb_first)
            gen0c = att_unit_gen(cur, 1, 0, osb_g1)
            # gen0c stops at 2 kpairs in-prep: its first AV (kpair 2) would
            # otherwise deadlock on gen0a's psV buffer before a's normalize
            gens = [(gen0a, [3, 3, 4, 5, 6, 7, 99, 99]),
                    (gen0b, [4, 4, 5, 6, 7, 99, 99, 99]),
                    (gen0c, [6, 7, 7, 7, 99, 99, 99, 99])]
            kk = [0, 0, 0]
            for d in range(ND):
                emit_prep_duo(cur)
                for gi, (gg, gt) in enumerate(gens):
                    while kk[gi] < 8 and d >= gt[kk[gi]]:
                        next(gg)
                        kk[gi] += 1
            finish_prep(cur)
            for d in range(4, ND):          # deferred V duos (ki groups 2-3)
                emit_v_duo(d, on_dve=True)
            for _ in gen0a:       # finish unit (0,0,0) incl. its normalize
                pass
            for _ in gen0b:       # finish unit (0,0,1)
                pass

            # ------------- main loop: 24 units, cross-unit pipelined -----------
            # Fillers (prep duos / o-transposes / proj tiles) are paced by a
            # PE-microsecond credit that accrues per kpair, matching the
            # ACT-gated PE idle (~0.4us/kpair) so prep spreads across the
            # whole pair phase instead of bunching in the first unit.
            fillers = []   # (closure, PE-cost-us) — prep duos, strict FIFO
            delayed = []   # (closure, cost, not_before_tick) — otr/proj; their
                           # PE ops wait on fresh DVE results, so entering the
                           # PE stream too early stalls it on the DVE queue
            credit = [0.0]
            ticks = [0]

            def drain_credit():
                while True:
                    if delayed and ticks[0] >= delayed[0][2] and credit[0] >= delayed[0][1]:
                        f, c, _ = delayed.pop(0)
                    elif fillers and credit[0] >= fillers[0][1]:
                        f, c = fillers.pop(0)
                    else:
                        return
                    f()
                    credit[0] -= c

            rate = [0.40]

            def tick(n=1):
                ticks[0] += 1
                credit[0] = min(credit[0] + rate[0] * n, 3.0)
                drain_credit()

            def g0_done():
                for qt in range(4):
                    delayed.append((lambda qt=qt: emit_otr(0, 0, osb_first, qt), 0.12, ticks[0] + 5))

            g0_done()
            pending = [(gen0c, None)]

            def run_unit(gen, after_cb=None):
                next(gen)                     # kpair 0: scores + exp only
                if pending[0] is not None:
                    pgen, pcb = pending[0]
                    for _ in pgen:            # prev unit: trailing AVs + norm
                        tick()
                    if pcb is not None:
                        pcb()
                pending[0] = (gen, after_cb)
                next(gen)                     # kpair 1 (still no AV: depth 2)
                for _ in range(6):
                    next(gen)
                    tick()

            def flush_pending():
                if pending[0] is not None:
                    pgen, pcb = pending[0]
                    for _ in pgen:
                        pass
                    if pcb is not None:
                        pcb()
                    pending[0] = None

            for p in range(3):
                rate[0] = 0.9 if p == 2 else 0.40
                nxt = new_pair_state(p + 1) if p < 2 else None
                if nxt is not None:
                    for d in range(ND):
                        fillers.append((lambda st=nxt: emit_prep_duo(st), 1.5))
                for g in range(NG):
                    if p == 0 and g == 0:
                        continue          # both g0 units ran gated in prep-0
                    if p == 0 and g == 1:
                        osb_t = osb_g1    # (0,1,0) partially ran gated
                    else:
                        osb_t = osb.tile([128, 4, 128], BF, tag="osb")
                    last = (p == 2 and g == NG - 1)

                    def unit_done(p=p, g=g, o=osb_t):
                        # enqueue only after the unit's normalize is emitted;
                        # hold back a few kpairs so the DVE queue drains first
                        for qt in range(4):
                            delayed.append((lambda qt=qt: emit_otr(p, g, o, qt), 0.12, ticks[0] + 5))
                        if p == 2:
                            for i in range(g * NG, (g + 1) * NG):
                                delayed.append((lambda i=i: emit_proj_tile(i), 0.96, ticks[0] + 8))

                    def qt_done_last(qt, o=osb_t, g=g):
                        # tail pipeline: transpose + project per query tile as
                        # soon as its normalize lands
                        emit_otr(2, g, o, qt)
                        emit_proj_tile(g * NG + qt, tail=True)

                    for hh in range(2):
                        if p == 0 and g == 1 and hh == 0:
                            continue      # gen0c already in pending
                        run_unit(att_unit_gen(cur, g, hh, osb_t,
                                              qt_done_last if (last and hh == 1) else None),
                                 unit_done if (hh == 1 and not last) else None)
                if nxt is not None:
                    while fillers:
                        fillers.pop(0)[0]()
                    finish_prep(nxt)
                    cur = nxt
            flush_pending()
            while fillers or delayed:
                if fillers:
                    fillers.pop(0)[0]()
                else:
                    delayed.pop(0)[0]()

    nc.compile()
    return nc


_NC = None


def _get_nc():
    global _NC
    if _NC is None:
        _NC = build_program()
    return _NC


def _prep_inputs(x, cos, sin, qkv_w, q_norm_w, k_norm_w, proj_w):
    bf = ml_dtypes.bfloat16
    cos2 = np.asarray(cos, np.float32).reshape(N, D // 2)
    sin2 = np.asarray(sin, np.float32).reshape(N, D // 2)
    cos_full = np.concatenate([cos2, cos2], axis=1)          # [N, 64]
    sin_signed = np.concatenate([-sin2, sin2], axis=1)       # [N, 64]

    def tables(w):
        w = np.asarray(w, np.float32)
        wswap = np.concatenate([w[D // 2 :], w[: D // 2]])
        cw = (8.0 * cos_full * w[None, :]).astype(np.float32)
        sw = (8.0 * sin_signed * wswap[None, :]).astype(np.float32)
        return np.ascontiguousarray(cw), np.ascontiguousarray(sw)

    cwq_, swq_ = tables(q_norm_w)
    cwk_, swk_ = tables(k_norm_w)
    cqk_ = np.ascontiguousarray(np.stack([cwq_, cwk_], axis=1).reshape(N, 2 * D))
    sqk_ = np.ascontiguousarray(np.stack([swq_, swk_], axis=1).reshape(N, 2 * D))

    in_maps = []
    for c in range(8):
        b, hg = c // 2, c % 2
        h0 = HPC * hg
        rows = np.r_[h0 * D : (h0 + HPC) * D]
        wq = qkv_w[rows]          # [384, C]
        wk = qkv_w[C + rows]
        wv = qkv_w[2 * C + rows]
        # pack as [q0|k0 (256), v (384), q1|k1, q2|k2]
        parts = [wq[0:128], wk[0:128], wv]
        for p in range(1, 3):
            parts.append(wq[p * 128 : (p + 1) * 128])
            parts.append(wk[p * 128 : (p + 1) * 128])
        wqkvT_ = np.ascontiguousarray(np.concatenate(parts, 0).T.astype(bf))
        projT_ = np.ascontiguousarray(proj_w[:, rows].T.astype(bf))
        xT_ = np.ascontiguousarray(x[b].T.astype(bf))
        in_maps.append({
            "xT": xT_, "wqkvT": wqkvT_, "projT": projT_,
            "cqk": cqk_, "sqk": sqk_,
        })
    return in_maps


def kernel(x, cos, sin, qkv_w, q_norm_w, k_norm_w, proj_w, proj_b, _want_trace=False):
    x = np.asarray(x, np.float32)
    qkv_w = np.asarray(qkv_w, np.float32)
    proj_w = np.asarray(proj_w, np.float32)
    proj_b = np.asarray(proj_b, np.float32)
    in_maps = _prep_inputs(x, cos, sin, qkv_w, q_norm_w, k_norm_w, proj_w)
    nc = _get_nc()
    res = run_bass_kernel_spmd(nc, in_maps, core_ids=list(range(8)), trace=_want_trace)
    out = np.empty((B, N, C), np.float32)
    for b in range(B):
        out[b] = (np.asarray(res.results[2 * b]["out"], np.float32)
                  + np.asarray(res.results[2 * b + 1]["out"], np.float32)
                  + proj_b[None, :])
    if _want_trace:
        return out, res
    return out
